# revision 1
# baseline (speedup 1.0000x reference)
"""GATv2 ChemAttentionBlock on 8 TRN2 NeuronCores.

Strategy (graph/data parallel, per sharding hint):
  - nodes partitioned into 8 contiguous ranges of 1250 (dst shard per core);
    edges (with self-loops) routed to the core owning their dst.
  - each core computes the full xl = x@Wl+bl table (replicated work, x input
    row-rotated per core so its own dst shard sits at rows [0,1280)) and its
    xr shard, writes them to HBM as bf16 gather tables.
  - edge phase: dma_gather rows of xl[src] / xr[dst] (SWDGE custom DMA),
    alpha = att . LeakyReLU(xl[src]+xr[dst]) computed via the identity
      att.LR(z) = 0.6*sum(s*v) + 0.4*sum(s*|v|),  v = |att|*z, s = sign(att)
    with columns permuted so positive-sign features are contiguous ->
    two tensor_tensor_reduce passes per 128-edge chunk, no edge-level
    transcendentals except one exp per edge.
  - segment softmax + aggregation fused: out[d] = sum_e w_e*xl[src_e] / sum w_e
    via PE matmuls with an indicator-weight matrix IndW[e,d] = w_e*(dst_e==d)
    built on DVE from iota/compare; denominator rides along as a constant
    "ones" column of the xl table. PSUM accumulates each 128-dst block.
  - epilogue: bias/relu/dropout per shard; BatchNorm stats via ones-vector
    matmuls + a 512-float AllReduce across the 8 cores.
Host-side work is layout only: sorting/sharding edges, padding, index
packing, weight column permutation/scaling, and final row/col unpermute.
"""

import os
import sys

for _p in ("/root/.axon_site", "/root/.axon_site/_ro/trn_rl_repo",
           "/root/.axon_site/_ro/pypackages"):
    if os.path.isdir(_p) and _p not in sys.path:
        sys.path.append(_p)

import numpy as np
import ml_dtypes

N, E, DIN, DOUT = 10000, 160000, 256, 256
NCORES = 8
NPC = N // NCORES            # nodes per core (1250)
B = 10                       # dst blocks of 128 per core
SPAD = B * 128               # padded shard rows (1280)
NT = 79                      # node tiles for the xl table (79*128 = 10112)
TABN = NT * 128
XLW = 384                    # table row width (256 feat + aug cols + pad)
AUGW = 258                   # written table cols: 256 feat + 2 aug
NEG_SLOPE = 0.2
BN_EPS = 1e-5

f32 = np.float32
bf16 = ml_dtypes.bfloat16


def _wrap_idx(flat_i16: np.ndarray) -> np.ndarray:
    """Pack a flat int16 index list into the [128, n//16] SWDGE layout:
    index i lives at [i % 16, i // 16], replicated across the 8 Q7 core
    partition groups (partitions 16k..16k+15)."""
    m16 = flat_i16.reshape(-1, 16).T        # [16, n/16]
    return np.tile(m16, (8, 1)).copy()      # [128, n/16]


def _host_prep(x, edge_index, Wl, bl, Wr, br, att, bias, gamma, beta, dropout_u):
    ei = np.asarray(edge_index).astype(np.int64)
    src_all = np.concatenate([ei[0], np.arange(N, dtype=np.int64)])
    dst_all = np.concatenate([ei[1], np.arange(N, dtype=np.int64)])
    order = np.argsort(dst_all, kind="stable")
    srcs = src_all[order]
    dsts = dst_all[order]
    bounds = np.searchsorted(dsts, np.arange(NCORES + 1) * NPC)

    # column permutation: positive-att features first, scale by |att|
    att = np.asarray(att, dtype=f32)
    perm = np.argsort(att < 0, kind="stable").astype(np.int64)
    P = int((att >= 0).sum())
    matt = np.maximum(np.abs(att[perm]), 1e-6).astype(f32)

    Wl_s = (np.asarray(Wl, f32)[:, perm] * matt).astype(f32)
    bl_s = (np.asarray(bl, f32)[perm] * matt).astype(f32)
    Wr_s = (np.asarray(Wr, f32)[:, perm] * matt).astype(f32)
    br_s = (np.asarray(br, f32)[perm] * matt).astype(f32)
    # augmented columns: col 256 -> xl: const 1 (denominator) / xr: SR row;
    # col 257 -> xl: SL row / xr: 0.  SL/SR = signed sums sum_pos - sum_neg.
    sgn = np.where(np.arange(DOUT) < P, 1.0, -1.0).astype(f32)
    Wl_aug = np.column_stack([Wl_s, np.zeros(DIN, f32), Wl_s @ sgn]).astype(f32)
    bl_aug = np.concatenate([bl_s, [1.0], [bl_s @ sgn]]).astype(f32)
    Wr_aug = np.column_stack([Wr_s, Wr_s @ sgn, np.zeros(DIN, f32)]).astype(f32)
    br_aug = np.concatenate([br_s, [br_s @ sgn], [0.0]]).astype(f32)

    x = np.asarray(x, f32)
    du = np.asarray(dropout_u, f32)

    cores = []
    cpb_needed = 1
    for k in range(NCORES):
        e0, e1 = bounds[k], bounds[k + 1]
        srck = srcs[e0:e1]
        dstk = dsts[e0:e1] - k * NPC         # local 0..NPC-1
        deg = np.bincount(dstk, minlength=NPC)

        # greedy LPT: pack nodes into B blocks (<=128 nodes each), balancing edges
        node_order = np.argsort(-deg, kind="stable")
        block_load = np.zeros(B, np.int64)
        block_cnt = np.zeros(B, np.int64)
        block_of = np.empty(NPC, np.int64)
        slot_of = np.empty(NPC, np.int64)
        for n in node_order:
            open_b = np.nonzero(block_cnt < 128)[0]
            b = open_b[np.argmin(block_load[open_b])]
            block_of[n] = b
            slot_of[n] = block_cnt[b]
            block_cnt[b] += 1
            block_load[b] += deg[n]
        cpb_needed = max(cpb_needed, int(np.ceil(block_load.max() / 128)))
        cores.append((k, srck, dstk, block_of, slot_of))

    CPB = cpb_needed
    C = B * CPB

    per_core = []
    for (k, srck, dstk, block_of, slot_of) in cores:
        eblk = block_of[dstk]
        o2 = np.argsort(eblk, kind="stable")
        src2 = srck[o2]
        dst2 = dstk[o2]
        eb2 = eblk[o2]
        starts = np.searchsorted(eb2, np.arange(B))
        pos_within = np.arange(len(eb2)) - starts[eb2]
        gpos = eb2 * (CPB * 128) + pos_within
        assert pos_within.max(initial=0) < CPB * 128

        # x row order: own shard in block-slot order at rows [0, SPAD),
        # all other nodes rotated after; src indices remapped accordingly.
        pos_of = np.empty(N, np.int64)
        own = np.arange(NPC)
        pos_of[k * NPC + own] = block_of * 128 + slot_of
        rest = (np.arange(N - NPC) + (k + 1) * NPC) % N     # global ids, rotated
        pos_of[rest] = SPAD + np.arange(N - NPC)
        g_src = np.zeros(C * 128, np.int16)
        g_off = np.full(C * 128, -1, np.int64)      # pad: no slot match
        g_src[gpos] = pos_of[src2].astype(np.int16)
        g_off[gpos] = slot_of[dst2]
        # one-hot patterns, shipped as bf16: ind0[e, c, d] / ind0t[d, c, e]
        offg = g_off.reshape(C, 128)                # [c, e]
        oh = (offg[:, :, None] == np.arange(128)[None, None, :])  # [c, e, d]
        ind0 = np.ascontiguousarray(
            oh.transpose(1, 0, 2)).reshape(128, C * 128).astype(bf16)
        ind0t = np.ascontiguousarray(
            oh.transpose(2, 0, 1)).reshape(128, C * 128).astype(bf16)

        x_rot = np.zeros((TABN, DIN), f32)
        x_rot[pos_of] = x

        rowmap = np.full(SPAD, -1, np.int64)
        rowmap[block_of * 128 + slot_of] = np.arange(NPC)
        valid = rowmap >= 0
        du_p = np.zeros((SPAD, DOUT), f32)
        du_p[valid] = du[k * NPC + rowmap[valid]][:, perm]

        per_core.append(dict(
            isrc=_wrap_idx(g_src),
            ind0=ind0,
            ind0t=ind0t,
            du=du_p,
            x=x_rot,
            rowmap=rowmap,
            valid=valid,
        ))

    shared = dict(
        wl=Wl_aug, wr=Wr_aug,
        blrep=np.tile(bl_aug, (128, 1)).astype(f32),
        brrep=np.tile(br_aug, (128, 1)).astype(f32),
        ident=np.eye(128, dtype=bf16),
        onescol=np.ones((128, 1), f32),
        onesrow=np.ones((1, 128), f32),
        biasrep=np.tile(np.asarray(bias, f32)[perm], (128, 1)).astype(f32),
        invmatt=np.tile((1.0 / matt).astype(f32), (128, 1)).astype(f32),
        gammarow=np.asarray(gamma, f32)[perm][None, :].copy(),
        betarow=np.asarray(beta, f32)[perm][None, :].copy(),
    )
    return per_core, shared, perm, P, CPB


def _build_program(P: int, CPB: int, ncores: int = NCORES):
    KSTAGE = int(os.environ.get("KSTAGE", "4"))  # 1=tables 2=+agg1blk 3=+allblks(no cc) 4=full
    import concourse.bass as bass
    import concourse.bacc as bacc
    import concourse.mybir as mybir
    from concourse.tile import TileContext

    dt = mybir.dt
    op = mybir.AluOpType
    act = mybir.ActivationFunctionType
    C = B * CPB

    nc = bacc.Bacc(None, debug=False, num_devices=NCORES)

    # I/O
    x_h = nc.dram_tensor("x", [TABN, DIN], dt.float32, kind="ExternalInput")
    wl_h = nc.dram_tensor("wl", [DIN, AUGW], dt.float32, kind="ExternalInput")
    wr_h = nc.dram_tensor("wr", [DIN, AUGW], dt.float32, kind="ExternalInput")
    blrep_h = nc.dram_tensor("blrep", [128, AUGW], dt.float32, kind="ExternalInput")
    brrep_h = nc.dram_tensor("brrep", [128, AUGW], dt.float32, kind="ExternalInput")
    ident_h = nc.dram_tensor("ident", [128, 128], dt.bfloat16, kind="ExternalInput")
    onescol_h = nc.dram_tensor("onescol", [128, 1], dt.float32, kind="ExternalInput")
    onesrow_h = nc.dram_tensor("onesrow", [1, 128], dt.float32, kind="ExternalInput")
    biasrep_h = nc.dram_tensor("biasrep", [128, DOUT], dt.float32, kind="ExternalInput")
    invmatt_h = nc.dram_tensor("invmatt", [128, DOUT], dt.float32, kind="ExternalInput")
    gamma_h = nc.dram_tensor("gammarow", [1, DOUT], dt.float32, kind="ExternalInput")
    beta_h = nc.dram_tensor("betarow", [1, DOUT], dt.float32, kind="ExternalInput")
    isrc_h = nc.dram_tensor("isrc", [128, C * 8], dt.int16, kind="ExternalInput")
    ind0_h = nc.dram_tensor("ind0", [128, C * 128], dt.bfloat16, kind="ExternalInput")
    ind0t_h = nc.dram_tensor("ind0t", [128, C * 128], dt.bfloat16,
                             kind="ExternalInput")
    du_h = nc.dram_tensor("du", [SPAD, DOUT], dt.float32, kind="ExternalInput")
    out_h = nc.dram_tensor("out", [SPAD, DOUT], dt.float32, kind="ExternalOutput")

    # internal DRAM
    xl_tab = nc.dram_tensor("xl_tab", [TABN, XLW], dt.bfloat16)
    cc_in = nc.dram_tensor("cc_in", [1, 2 * DOUT], dt.float32)
    cc_out = nc.dram_tensor("cc_out", [1, 2 * DOUT], dt.float32, addr_space="Shared")

    with TileContext(nc) as tc:
        with tc.tile_pool(name="const", bufs=1) as cpool:
            zeros_sb = cpool.tile([128, 128], dt.bfloat16, tag="zeros")
            nc.vector.memset(zeros_sb[:], 0.0)
            xr_keep = cpool.tile([128, B, AUGW], dt.bfloat16, tag="xrkeep")
            biasrep_sb = cpool.tile([128, DOUT], dt.float32, tag="biasrep")
            nc.sync.dma_start(out=biasrep_sb[:], in_=biasrep_h[:, :])
            invmatt_sb = cpool.tile([128, DOUT], dt.float32, tag="invmatt")
            nc.sync.dma_start(out=invmatt_sb[:], in_=invmatt_h[:, :])
            onescol_sb = cpool.tile([128, 1], dt.float32, tag="onescol")
            nc.sync.dma_start(out=onescol_sb[:], in_=onescol_h[:, :])
            onesrow_sb = cpool.tile([1, 128], dt.float32, tag="onesrow")
            nc.sync.dma_start(out=onesrow_sb[:], in_=onesrow_h[:, :])
            gamma_sb = cpool.tile([1, DOUT], dt.float32, tag="gamma")
            nc.sync.dma_start(out=gamma_sb[:], in_=gamma_h[:, :])
            beta_sb = cpool.tile([1, DOUT], dt.float32, tag="beta")
            nc.sync.dma_start(out=beta_sb[:], in_=beta_h[:, :])
            du_sb = cpool.tile([128, B, DOUT], dt.float32, tag="du")
            nc.sync.dma_start(
                out=du_sb[:], in_=du_h[:, :].rearrange("(b p) c -> p b c", p=128))
            out_keep = cpool.tile([128, B, DOUT], dt.float32, tag="okeep")
            bn_acc = cpool.tile([1, 2 * DOUT], dt.float32, tag="bnacc")
            nc.vector.memset(bn_acc[:], 0.0)

            # ---------------- phase A: build xl/xr tables ----------------
            with tc.tile_pool(name="pa", bufs=1) as pa, \
                 tc.tile_pool(name="pa_ps", bufs=2, space=bass.MemorySpace.PSUM) as tpps, \
                 tc.tile_pool(name="mm_ps", bufs=4, space=bass.MemorySpace.PSUM) as mmps:
                x_bf = nc.dram_tensor("x_bf", [TABN, DIN], dt.bfloat16)
                nc.gpsimd.dma_start(out=x_bf[:, :], in_=x_h[:, :])
                wl_sb = pa.tile([128, 2, AUGW], dt.bfloat16, tag="wl")
                nc.gpsimd.dma_start(
                    out=wl_sb[:], in_=wl_h[:, :].rearrange("(h p) o -> p h o", p=128))
                wr_sb = pa.tile([128, 2, AUGW], dt.bfloat16, tag="wr")
                nc.gpsimd.dma_start(
                    out=wr_sb[:], in_=wr_h[:, :].rearrange("(h p) o -> p h o", p=128))
                blrep_sb = pa.tile([128, AUGW], dt.float32, tag="blrep")
                nc.sync.dma_start(out=blrep_sb[:], in_=blrep_h[:, :])
                brrep_sb = pa.tile([128, AUGW], dt.float32, tag="brrep")
                nc.sync.dma_start(out=brrep_sb[:], in_=brrep_h[:, :])
                xt_sb = pa.tile([128, 2, TABN], dt.bfloat16, tag="xt")
                for h in range(2):
                    nc.sync.dma_start(
                        out=xt_sb[:, h, :],
                        in_=x_bf[:, h * 128:(h + 1) * 128], transpose=True)

                xl_sb = pa.tile([128, NT, XLW], dt.bfloat16, tag="xltab")
                for t in range(NT):
                    pxl = mmps.tile([128, AUGW], dt.float32, tag="mm")
                    nc.tensor.matmul(
                        pxl[:], xt_sb[:, 0, t * 128:(t + 1) * 128], wl_sb[:, 0, :],
                        start=True, stop=False)
                    nc.tensor.matmul(
                        pxl[:], xt_sb[:, 1, t * 128:(t + 1) * 128], wl_sb[:, 1, :],
                        start=False, stop=True)
                    nc.vector.tensor_add(xl_sb[:, t, 0:AUGW], pxl[:], blrep_sb[:])

                for t in range(B):
                    pxr = mmps.tile([128, AUGW], dt.float32, tag="mm")
                    nc.tensor.matmul(
                        pxr[:], xt_sb[:, 0, t * 128:(t + 1) * 128], wr_sb[:, 0, :],
                        start=True, stop=False)
                    nc.tensor.matmul(
                        pxr[:], xt_sb[:, 1, t * 128:(t + 1) * 128], wr_sb[:, 1, :],
                        start=False, stop=True)
                    nc.vector.tensor_add(xr_keep[:, t, 0:AUGW], pxr[:], brrep_sb[:])

                nc.sync.dma_start(
                    out=xl_tab[:, :].rearrange("(t p) c -> p t c", p=128),
                    in_=xl_sb[:])

            # ---------------- phase B: edge phase ----------------
            with tc.tile_pool(name="pb", bufs=3) as pb, \
                 tc.tile_pool(name="pbs", bufs=3) as pbs, \
                 tc.tile_pool(name="agg_ps", bufs=2, space=bass.MemorySpace.PSUM) as aggps, \
                 tc.tile_pool(name="vx_ps", bufs=3, space=bass.MemorySpace.PSUM) as vxps, \
                 tc.tile_pool(name="bn_ps", bufs=2, space=bass.MemorySpace.PSUM) as bnps:
                isrc_sb = pb.tile([128, C * 8], dt.int16, tag="isrc")
                nc.sync.dma_start(out=isrc_sb[:], in_=isrc_h[:, :])

                nidx = CPB * 128
                nblocks = 0 if KSTAGE < 2 else (1 if KSTAGE == 2 else B)
                for b in range(nblocks):
                    xlg = pb.tile([128, CPB, XLW], dt.bfloat16, tag="xlg")
                    # SWDGE gathers above ~2k indices overflow the descriptor
                    # ring; split into <=8-chunk (1024-index) calls.
                    GMAX = 8
                    for g0 in range(0, CPB, GMAX):
                        gc = min(GMAX, CPB - g0)
                        gn = gc * 128
                        io = (b * CPB + g0) * 8
                        nc.gpsimd.dma_gather(
                            xlg[:, g0:g0 + gc, :], xl_tab[:, :],
                            isrc_sb[:, io:io + gc * 8], gn, gn, XLW)
                    ind0_sb = pb.tile([128, CPB, 128], dt.bfloat16, tag="ind0")
                    nc.sync.dma_start(
                        out=ind0_sb[:],
                        in_=ind0_h[:, b * CPB * 128:(b + 1) * CPB * 128])
                    ind0t_sb = pb.tile([128, CPB, 128], dt.bfloat16, tag="ind0t")
                    nc.sync.dma_start(
                        out=ind0t_sb[:],
                        in_=ind0t_h[:, b * CPB * 128:(b + 1) * CPB * 128])

                    v = pb.tile([128, CPB, DOUT], dt.bfloat16, tag="v")
                    junk = pb.tile([128, CPB, DOUT], dt.bfloat16, tag="junk")
                    aparts = pb.tile([128, 2, CPB], dt.float32, tag="aparts")
                    for c in range(CPB):
                        # xr row expansion: vxr[e, :] = xr_keep[slot(e), b, :]
                        vxr = vxps.tile([128, DOUT], dt.float32, tag="vxr")
                        nc.tensor.matmul(
                            vxr[:], ind0t_sb[:, c, :], xr_keep[:, b, 0:DOUT],
                            start=True, stop=True)
                        nc.vector.tensor_tensor(
                            v[:, c, :], xlg[:, c, 0:DOUT], vxr[:, 0:DOUT], op.add)
                        nc.scalar.activation(
                            junk[:, c, 0:P], v[:, c, 0:P], act.Relu,
                            accum_out=aparts[:, 0, c:c + 1])
                        nc.vector.tensor_scalar(
                            junk[:, c, P:DOUT], v[:, c, P:DOUT], 0.0, None,
                            op.max, op.add, accum_out=aparts[:, 1, c:c + 1])

                    t2 = pbs.tile([128, CPB], dt.float32, tag="t2")
                    nc.vector.tensor_tensor(t2[:], aparts[:, 0, :], aparts[:, 1, :],
                                            op.subtract)
                    pre = pbs.tile([128, CPB], dt.float32, tag="pre")
                    nc.vector.scalar_tensor_tensor(
                        out=pre[:], in0=t2[:], scalar=4.0,
                        in1=xlg[:, :, AUGW - 1], op0=op.mult, op1=op.add)
                    w_t = pbs.tile([128, CPB], dt.float32, tag="w")
                    nc.scalar.activation(w_t[:], pre[:], act.Exp, scale=0.2)

                    indw = pbs.tile([128, CPB, 128], dt.bfloat16, tag="indw")
                    for c in range(CPB):
                        nc.vector.scalar_tensor_tensor(
                            out=indw[:, c, :], in0=ind0_sb[:, c, :],
                            scalar=w_t[:, c:c + 1], in1=zeros_sb[:],
                            op0=op.mult, op1=op.add)

                    ps_agg = aggps.tile([128, DOUT + 1], dt.float32, tag="agg")
                    for c in range(CPB):
                        nc.tensor.matmul(
                            ps_agg[:], indw[:, c, :], xlg[:, c, 0:DOUT + 1],
                            start=(c == 0), stop=(c == CPB - 1))

                    # epilogue for block b
                    den_s = pbs.tile([128, 1], dt.float32, tag="dens")
                    nc.vector.tensor_scalar_add(
                        den_s[:], ps_agg[:, DOUT:DOUT + 1], 1e-30)
                    rec = pbs.tile([128, 1], dt.float32, tag="rec")
                    nc.vector.reciprocal(rec[:], den_s[:])
                    o1 = pbs.tile([128, DOUT], dt.float32, tag="o1")
                    nc.vector.scalar_tensor_tensor(
                        out=o1[:], in0=ps_agg[:, 0:DOUT], scalar=rec[:],
                        in1=invmatt_sb[:], op0=op.mult, op1=op.mult)
                    nc.vector.tensor_add(o1[:], o1[:], biasrep_sb[:])
                    nc.vector.tensor_scalar_max(o1[:], o1[:], 0.0)
                    maskt = pbs.tile([128, DOUT], dt.float32, tag="mask")
                    nc.vector.tensor_scalar(
                        maskt[:], du_sb[:, b, :], 0.5, None, op.is_ge)
                    nc.vector.tensor_tensor(
                        out_keep[:, b, :], o1[:], maskt[:], op.mult)
                    sq_t = pbs.tile([128, DOUT], dt.float32, tag="sq")
                    nc.scalar.square(sq_t[:], out_keep[:, b, :])
                    ps_bn = bnps.tile([1, 2 * DOUT], dt.float32, tag="bn")
                    nc.tensor.matmul(ps_bn[:, 0:DOUT], onescol_sb[:],
                                     out_keep[:, b, :], start=True, stop=True)
                    nc.tensor.matmul(ps_bn[:, DOUT:2 * DOUT], onescol_sb[:],
                                     sq_t[:], start=True, stop=True)
                    nc.vector.tensor_add(bn_acc[:], bn_acc[:], ps_bn[:])

            # ---------------- BN finalize + AllReduce ----------------
            with tc.tile_pool(name="pc", bufs=1) as pc, \
                 tc.tile_pool(name="pc_ps", bufs=2, space=bass.MemorySpace.PSUM) as pcps:
                if KSTAGE <= 2:
                    # debug: dump out_keep for processed blocks, skip BN
                    for b in range(nblocks):
                        dbg = pc.tile([128, DOUT], dt.float32, tag="dbg")
                        nc.vector.tensor_copy(dbg[:], out_keep[:, b, :])
                        nc.sync.dma_start(
                            out=out_h[b * 128:(b + 1) * 128, :], in_=dbg[:])
                    finalize_bn = False
                elif KSTAGE == 3:
                    bn_tot = pc.tile([1, 2 * DOUT], dt.float32, tag="bntot")
                    nc.vector.tensor_copy(bn_tot[:], bn_acc[:])
                    finalize_bn = True
                else:
                    nc.sync.dma_start(out=cc_in[:, :], in_=bn_acc[:])
                    nc.gpsimd.collective_compute(
                        "AllReduce", op.add,
                        replica_groups=[list(range(ncores))],
                        ins=[cc_in[:, :]], outs=[cc_out[:, :]])
                    bn_tot = pc.tile([1, 2 * DOUT], dt.float32, tag="bntot")
                    nc.sync.dma_start(out=bn_tot[:], in_=cc_out[:, :])
                    finalize_bn = True

                if finalize_bn:
                    mean = pc.tile([1, DOUT], dt.float32, tag="mean")
                    nc.vector.tensor_scalar_mul(mean[:], bn_tot[:, 0:DOUT], 1.0 / N)
                    ex2 = pc.tile([1, DOUT], dt.float32, tag="ex2")
                    nc.vector.tensor_scalar_mul(ex2[:], bn_tot[:, DOUT:2 * DOUT],
                                                1.0 / N)
                    msq = pc.tile([1, DOUT], dt.float32, tag="msq")
                    nc.vector.tensor_tensor(msq[:], mean[:], mean[:], op.mult)
                    var = pc.tile([1, DOUT], dt.float32, tag="var")
                    nc.vector.tensor_tensor(var[:], ex2[:], msq[:], op.subtract)
                    nc.vector.tensor_scalar_add(var[:], var[:], BN_EPS / 4.0)
                    sd = pc.tile([1, DOUT], dt.float32, tag="sd")
                    nc.scalar.sqrt(sd[:], var[:])
                    rs = pc.tile([1, DOUT], dt.float32, tag="rs")
                    nc.vector.reciprocal(rs[:], sd[:])

                    ab = pc.tile([1, 2 * DOUT], dt.float32, tag="ab")
                    nc.vector.tensor_tensor(ab[:, 0:DOUT], gamma_sb[:], rs[:],
                                            op.mult)
                    tmpm = pc.tile([1, DOUT], dt.float32, tag="tmpm")
                    nc.vector.tensor_tensor(tmpm[:], ab[:, 0:DOUT], mean[:], op.mult)
                    nc.vector.tensor_tensor(ab[:, DOUT:2 * DOUT], beta_sb[:],
                                            tmpm[:], op.subtract)
                    ps_ab = pcps.tile([128, 2 * DOUT], dt.float32, tag="ab")
                    nc.tensor.matmul(ps_ab[:], onesrow_sb[:], ab[:],
                                     start=True, stop=True)

                    for b in range(B):
                        tt = pc.tile([128, DOUT], dt.float32, tag="fin")
                        nc.vector.tensor_tensor(tt[:], out_keep[:, b, :],
                                                ps_ab[:, 0:DOUT], op.mult)
                        nc.vector.tensor_add(tt[:], tt[:], ps_ab[:, DOUT:2 * DOUT])
                        nc.sync.dma_start(
                            out=out_h[b * 128:(b + 1) * 128, :], in_=tt[:])

    nc.finalize()
    return nc


def kernel(x, edge_index, Wl, bl, Wr, br, att, bias, gamma, beta, dropout_u,
           _trace=False, _ncores=NCORES):
    per_core, shared, perm, P, CPB = _host_prep(
        x, edge_index, Wl, bl, Wr, br, att, bias, gamma, beta, dropout_u)

    nc = _build_program(P, CPB, _ncores)

    in_maps = []
    for k in range(_ncores):
        pc = per_core[k]
        m = dict(
            x=pc["x"], wl=shared["wl"], wr=shared["wr"],
            blrep=shared["blrep"], brrep=shared["brrep"],
            ident=shared["ident"],
            onescol=shared["onescol"], onesrow=shared["onesrow"],
            biasrep=shared["biasrep"], invmatt=shared["invmatt"],
            gammarow=shared["gammarow"], betarow=shared["betarow"],
            isrc=pc["isrc"], ind0=pc["ind0"], ind0t=pc["ind0t"],
            du=pc["du"],
        )
        in_maps.append(m)

    from concourse.bass_utils import run_bass_kernel_spmd
    res = run_bass_kernel_spmd(nc, in_maps, core_ids=list(range(_ncores)),
                               trace=_trace)

    out_p = np.empty((N, DOUT), f32)
    for k in range(_ncores):
        shard = res.results[k]["out"]
        rowmap = per_core[k]["rowmap"]
        valid = per_core[k]["valid"]
        out_p[k * NPC + rowmap[valid]] = shard[valid]
    final = np.empty((N, DOUT), f32)
    final[:, perm] = out_p
    if _trace:
        kernel._last_exec_ns = res.exec_time_ns
        kernel._last_results = res
    return final

